# revision 28
# baseline (speedup 1.0000x reference)
"""Butterfly transform (12 layers, DIM=4096) on 8 TRN2 NeuronCores.

Math: the 12-layer butterfly factors exactly as
    W = (G  kron I_128) . blockdiag(M_0..M_31)
where M_t (128x128, dense) is the product of layers 0..6 restricted to
feature-tile t, and G (32x32) is the product of layers 7..11 acting on the
tile index alone (those layers pair features at strides >=128, so each
(a,b) scalar applies uniformly to a whole 128-feature tile).

Device pipeline per 128-row batch chunk (feature-major compute):
  1. DMA x rows in natural [b, d] layout (contiguous 16KB/partition).
  2. PE transpose each 128x128 tile -> [f, b] layout.
  3. Stage A: matmul z_t = M_t @ x_t^T per tile (dense, layers 0..6).
  4. SBUF->SBUF DMA repartition into U[(q,t), (r,b)] with f = 32q + r:
     partitions enumerate (q, tile) pairs for fixed r.
  5. Stage B: matmul with the *data* as stationary operand and
     WB = I_4 kron G^T as moving operand: out[b, (q,t')] lands directly in
     batch-major partitions (layers 7..11 + the output transpose in one op).
  6. Engine copy PSUM->SBUF scatters (q,t') columns to natural d order.
  7. DMA contiguous rows back to HBM.

Sharding: pure data parallel over batch (8192 rows -> 1024/core),
parameters replicated.
"""

import os
import sys

import numpy as np

for _p in ("/opt/trn_rl_repo",):
    if _p not in sys.path and os.path.isdir(_p):
        sys.path.insert(0, _p)

import concourse.bass as bass
import concourse.tile as tile
from concourse import mybir
from concourse.bass_utils import run_bass_kernel_spmd
from concourse.vector_clock import ScopedClock


class _TileContext(tile.TileContext):
    """TileContext that caps sync waits per instruction.

    The walrus build in this container enforces 1 sync wait per regular
    instruction (2 for EventSemaphore); stock Tile sem-assignment can attach
    several. Hoist the extras onto standalone EventSemaphore instructions
    inserted immediately before the over-subscribed instruction on the same
    engine.
    """

    def _split_excess_waits(self, insts: list) -> list:
        nc = self.nc
        out = []
        for inst in insts:
            si = inst.sync_info
            waits = list(si.on_wait) if si else []
            cap = 2 if inst.opcode == "EventSemaphore" else 1
            if len(waits) > cap:
                extras, keep = waits[:-cap], waits[-cap:]
                for k in range(0, len(extras), 2):
                    ev = mybir.InstEventSemaphore(
                        name=nc.get_next_instruction_name(),
                        engine=inst.engine,
                        sync_info=mybir.SyncInfo(
                            on_wait=extras[k : k + 2], on_update=[]
                        ),
                        debug=inst.debug,
                    )
                    nc.register_instruction(ev)
                    out.append(ev)
                inst.sync_info = mybir.SyncInfo(
                    on_wait=keep, on_update=list(si.on_update)
                )
            out.append(inst)
        return out

    def _lower_ordered_insts(self, ordered):
        for name in list(ordered.keys()):
            ordered[name] = self._split_excess_waits(ordered[name])
        return super()._lower_ordered_insts(ordered)

    def _drain_and_barrier(self, tick_clock, wait_clock):
        nc = self.nc
        drain_inst = nc.sync.drain()
        wait_clock.add_sem_waits(
            drain_inst.ins, ScopedClock({None: tick_clock.global_clock})
        )
        si = drain_inst.ins.sync_info
        waits = list(si.on_wait) if si else []
        ups = list(si.on_update) if si else []
        if len(waits) > 1:
            num2sem = {h.num: h for h in self.sems.allocated().values()}
            drain_inst.ins.sync_info = mybir.SyncInfo(on_wait=waits[:1], on_update=ups)
            for w in waits[1:]:
                nc.sync.wait_ge(num2sem[w.id], w.wait_value)
        nc.all_engine_barrier()
        assert self.sems is not None
        popped = nc._tile_sem_poison_stack.pop()
        assert popped is self._sem_poison
        nc.clear_and_free_semaphores(list(self.sems.allocated().values()))
        nc.all_engine_barrier()

DIM = 4096
TILE = 128
NT = DIM // TILE  # 32 feature tiles
BATCH = 8192
NCORES = 8
BC = BATCH // NCORES  # 1024 rows per core
NSUB = 32  # r/q split: f = 32*q + r
NQ = TILE // NSUB  # 4

F32 = mybir.dt.float32


def _host_factor(a_flat: np.ndarray, b_flat: np.ndarray):
    """Build M_t (32x[128x128], layers 0..6 per tile) and G (32x32, layers 7..11)."""
    a_flat = np.asarray(a_flat, dtype=np.float32)
    b_flat = np.asarray(b_flat, dtype=np.float32)

    # M_t: apply layers 0..6 to the identity, restricted to tile t.
    M = np.zeros((NT, TILE, TILE), dtype=np.float32)
    for t in range(NT):
        xloc = np.eye(TILE, dtype=np.float32)  # rows = c_in basis
        off = 0
        for layer in range(7):
            bs = 1 << layer
            nb_global = DIM // (2 * bs)
            nb_local = TILE // (2 * bs)
            a_l = a_flat[off + t * nb_local : off + (t + 1) * nb_local]
            b_l = b_flat[off + t * nb_local : off + (t + 1) * nb_local]
            off += nb_global
            xv = xloc.reshape(TILE, nb_local, 2, bs)
            x0 = xv[:, :, 0, :]
            x1 = xv[:, :, 1, :]
            top = a_l[None, :, None] * x0 + b_l[None, :, None] * x1
            bot = -b_l[None, :, None] * x0 + a_l[None, :, None] * x1
            xloc = np.stack([top, bot], axis=2).reshape(TILE, TILE)
        M[t] = xloc.T  # xloc[c_in, c_out] -> M[t][c_out, c_in]

    # G: layers 7..11 on the 32-dim tile index.
    off = sum(DIM // (2 * (1 << l)) for l in range(7))
    G = np.eye(NT, dtype=np.float32)
    for layer in range(7, 12):
        bs = 1 << layer
        nb = DIM // (2 * bs)
        sigma = bs // TILE
        a_l = a_flat[off : off + nb]
        b_l = b_flat[off : off + nb]
        off += nb
        R = np.zeros((NT, NT), dtype=np.float32)
        for n in range(nb):
            for jj in range(sigma):
                t0 = n * 2 * sigma + jj
                t1 = t0 + sigma
                R[t0, t0] = a_l[n]
                R[t0, t1] = b_l[n]
                R[t1, t0] = -b_l[n]
                R[t1, t1] = a_l[n]
        G = R @ G

    # Device-side arrays.
    mts = np.ascontiguousarray(np.transpose(M, (2, 0, 1)))  # [c_in, t, c_out]
    # q-major interleave: partition p = 32q + t, so WB = I_4 kron G^T.
    # q-major spreads each shuffle's 4 destination partitions {t, t+32,
    # t+64, t+96} across 4 SBUF port groups (vs 1 for consecutive
    # partitions), quadrupling S2M write parallelism per shuffle DMA.
    # Zero-padded to [128, 256] (moving dim >= 256 keeps options open).
    wb = np.zeros((TILE, 2 * TILE), dtype=np.float32)
    wb[:, :TILE] = np.kron(np.eye(NQ, dtype=np.float32), G.T)
    return mts, wb


def build_nc(bc: int = BC, loop: int = 1, mm_dtype: str = "float32", stage: str = "full") -> bass.Bass:
    """Build the per-core Bass program for bc rows (bc % 256 == 0).

    loop > 1 wraps the whole pipeline in a hardware For_i that reprocesses
    the same input `loop` times — used only for wall-clock benchmarking
    (slope vs loop count cancels dispatch overhead).
    """
    assert bc % 256 == 0
    nsc = bc // 256  # superchunks of 256 rows (2 x 128-row j-chunks)

    mmdt = getattr(mybir.dt, mm_dtype)
    nc = bass.Bass()
    x_d = nc.dram_tensor("x", [bc, DIM], F32, kind="ExternalInput")
    mts_d = nc.dram_tensor("mts", [TILE, NT, TILE], mmdt, kind="ExternalInput")
    wb_d = nc.dram_tensor("wb", [TILE, 2 * TILE], mmdt, kind="ExternalInput")
    id_d = nc.dram_tensor("ident", [TILE, TILE], F32, kind="ExternalInput")
    y_d = nc.dram_tensor("y", [bc, DIM], F32, kind="ExternalOutput")

    with _TileContext(nc) as tc:
        with (
            tc.tile_pool(name="const", bufs=1) as constp,
            tc.tile_pool(name="xin", bufs=2) as xp,
            tc.tile_pool(name="z0", bufs=2) as z0p,
            tc.tile_pool(name="zc", bufs=4) as zcp,
            tc.tile_pool(name="upool", bufs=1) as up,
            tc.tile_pool(name="yout", bufs=2) as yp,
            tc.tile_pool(name="pst", bufs=3, space="PSUM") as pstp,
            tc.tile_pool(name="psa", bufs=3, space="PSUM") as psap,
            tc.tile_pool(name="psb", bufs=2, space="PSUM") as psbp,
        ):
            mts = constp.tile([TILE, NT, TILE], mmdt)
            nc.sync.dma_start(mts[:], mts_d[:])
            wb = constp.tile([TILE, 2 * TILE], mmdt)
            nc.sync.dma_start(wb[:], wb_d[:])
            ident = constp.tile([TILE, TILE], F32)
            nc.sync.dma_start(ident[:], id_d[:])

            engines = [nc.vector, nc.scalar]
            ecnt = 0

            def copy(dst, src):
                nonlocal ecnt
                e = engines[ecnt % 2]
                ecnt += 1
                if e is nc.vector:
                    e.tensor_copy(dst, src)
                else:
                    e.copy(dst, src)

            def _pipeline():
                for s in range(nsc):
                    _superchunk(s)

            def _superchunk(s):
                z0 = z0p.tile([TILE, NT, 256], mmdt, tag="z0")
                u = up.tile([TILE, NT, 256], mmdt, tag="u")

                # ---- load + transpose into z0 [f, t, b] ----
                for j in range(2):
                    row0 = (s * 2 + j) * TILE
                    xt = xp.tile([TILE, DIM], F32, tag="x")
                    nc.sync.dma_start(xt[:], x_d[row0 : row0 + TILE, :])
                    for tg in range(8):
                        pst = pstp.tile([TILE, 512], F32, tag="pst")
                        for u4 in range(4):
                            t = tg * 4 + u4
                            nc.tensor.transpose(
                                pst[:, u4 * TILE : (u4 + 1) * TILE],
                                xt[:, t * TILE : (t + 1) * TILE],
                                ident[:],
                            )
                        copy(
                            z0[:, tg * 4 : (tg + 1) * 4, j * TILE : (j + 1) * TILE],
                            pst[:].rearrange("p (t b) -> p t b", t=4),
                        )

                # ---- stage A: z_t = M_t @ z0_t, then repartition into u ----
                if stage == "t":
                    return
                for tp in range(16):
                    psa = psap.tile([TILE, 512], F32, tag="psa")
                    for u2 in range(2):
                        t = tp * 2 + u2
                        nc.tensor.matmul(
                            psa[:, u2 * 256 : (u2 + 1) * 256],
                            mts[:, t, :],
                            z0[:, t, :],
                            start=True,
                            stop=True,
                        )
                    zc = zcp.tile([TILE, 512], mmdt, tag="zc")
                    # zc always on DVE: ACT's HWDGE ring issues half the
                    # shuffles, and an ACT-queued producer copy behind an
                    # ACT-ring shuffle issue would deadlock.
                    nc.vector.tensor_copy(zc[:], psa[:])
                    if stage == "a":
                        continue
                    shuf_ring = nc.sync if tp % 2 == 0 else nc.scalar
                    for u2 in range(2):
                        t = tp * 2 + u2
                        # src [128=(q,r), 256=b] streams in (q, r, b) order;
                        # dst [4 partitions=q, (r, b)] consumes the same order.
                        src = zc[:, u2 * 256 : (u2 + 1) * 256]
                        dst = u[t::NT, :, :]
                        shuf_ring.dma_start(dst, src)

                # ---- stage B: y[b, (q,t')] = sum_p U[p, r, b] * WB[p, (q,t')] ----
                if stage in ("a", "shuf"):
                    return
                for j in range(2):
                    yt = yp.tile([TILE, DIM], F32, tag="y")
                    # cols n = 32*q'' + t'' scatter to d = 128*t'' + 32*q'' + r
                    ytv = yt[:].rearrange("p (t q r) -> p r q t", t=NT, q=NQ)
                    for rp in range(16):
                        psb = psbp.tile([TILE, 512], F32, tag="psb")
                        for v in range(2):
                            r = rp * 2 + v
                            nc.tensor.matmul(
                                psb[:, v * 256 : (v + 1) * 256],
                                u[:, r, j * TILE : (j + 1) * TILE],
                                wb[:],
                                start=True,
                                stop=True,
                            )
                        copy(
                            ytv[:, rp * 2 : (rp + 1) * 2, :, :],
                            psb[:].rearrange("p (v x) -> p v x", v=2)[
                                :, :, 0:TILE
                            ].rearrange("p v (q t) -> p v q t", q=NQ),
                        )
                    row0 = (s * 2 + j) * TILE
                    # ACT's HWDGE ring: y-store waits must not block x-loads.
                    nc.scalar.dma_start(y_d[row0 : row0 + TILE, :], yt[:])

            if loop > 1:
                with tc.For_i(0, loop, 1):
                    _pipeline()
            else:
                _pipeline()

    nc.finalize()
    return nc


_NC_CACHE: dict[int, bass.Bass] = {}


def kernel(x: np.ndarray, a_flat: np.ndarray, b_flat: np.ndarray) -> np.ndarray:
    x = np.ascontiguousarray(np.asarray(x, dtype=np.float32))
    assert x.shape == (BATCH, DIM)
    mts, wb = _host_factor(a_flat, b_flat)
    ident = np.eye(TILE, dtype=np.float32)

    if BC not in _NC_CACHE:
        _NC_CACHE[BC] = build_nc(BC)
    nc = _NC_CACHE[BC]

    in_maps = [
        {
            "x": np.ascontiguousarray(x[i * BC : (i + 1) * BC]),
            "mts": mts,
            "wb": wb,
            "ident": ident,
        }
        for i in range(NCORES)
    ]
    res = run_bass_kernel_spmd(nc, in_maps, list(range(NCORES))).results
    return np.concatenate([res[i]["y"] for i in range(NCORES)], axis=0)


def make_runner(nc: bass.Bass, in_maps: list[dict]):
    """Build a reusable jitted 8-core runner (no donation) for benchmarking.

    Returns (fn, dev_args, out_names, out_shapes); call fn(*dev_args) and
    block_until_ready. Outputs come back concatenated along axis 0.
    """
    import jax
    from jax.sharding import Mesh, NamedSharding, PartitionSpec
    from jax.experimental.shard_map import shard_map

    from concourse import bass2jax

    bass2jax.install_neuronx_cc_hook()
    assert nc.dbg_addr is None
    partition_name = nc.partition_id_tensor.name if nc.partition_id_tensor else None

    in_names, out_names, out_avals, zero_outs = [], [], [], []
    for alloc in nc.m.functions[0].allocations:
        if not isinstance(alloc, mybir.MemoryLocationSet):
            continue
        name = alloc.memorylocations[0].name
        if alloc.kind == "ExternalInput":
            if name != partition_name:
                in_names.append(name)
        elif alloc.kind == "ExternalOutput":
            out_names.append(name)
            shape = tuple(alloc.tensor_shape)
            dtype = mybir.dt.np(alloc.dtype)
            out_avals.append(jax.core.ShapedArray(shape, dtype))
            zero_outs.append(np.zeros(shape, dtype))
    n_params = len(in_names)
    in_names = in_names + out_names
    if partition_name is not None:
        in_names.append(partition_name)

    def _body(*args):
        operands = list(args)
        if partition_name is not None:
            operands.append(bass2jax.partition_id_tensor())
        outs = bass2jax._bass_exec_p.bind(
            *operands,
            out_avals=tuple(out_avals),
            in_names=tuple(in_names),
            out_names=tuple(out_names),
            lowering_input_output_aliases=(),
            sim_require_finite=True,
            sim_require_nnan=True,
            nc=nc,
        )
        return tuple(outs)

    devices = jax.devices()[:NCORES]
    mesh = Mesh(np.asarray(devices), ("core",))
    spec = PartitionSpec("core")
    fn = jax.jit(
        shard_map(
            _body,
            mesh=mesh,
            in_specs=(spec,) * (n_params + len(out_names)),
            out_specs=(spec,) * len(out_names),
            check_rep=False,
        ),
        keep_unused=True,
    )
    sharding = NamedSharding(mesh, spec)
    concat_in = [
        np.concatenate([np.asarray(m[name]) for m in in_maps], axis=0)
        for name in in_names[:n_params]
    ]
    concat_zeros = [
        np.zeros((NCORES * z.shape[0], *z.shape[1:]), z.dtype) for z in zero_outs
    ]
    dev_args = [jax.device_put(a, sharding) for a in concat_in + concat_zeros]
    return fn, dev_args, out_names, [a.shape for a in out_avals]


def build_null_nc() -> bass.Bass:
    """Tiny passthrough kernel to measure fixed dispatch/roundtrip overhead."""
    nc = bass.Bass()
    x_d = nc.dram_tensor("nx", [TILE, TILE], F32, kind="ExternalInput")
    y_d = nc.dram_tensor("ny", [TILE, TILE], F32, kind="ExternalOutput")
    with _TileContext(nc) as tc:
        with tc.tile_pool(name="p", bufs=1) as p:
            t = p.tile([TILE, TILE], F32)
            nc.sync.dma_start(t[:], x_d[:])
            nc.sync.dma_start(y_d[:], t[:])
    nc.finalize()
    return nc


def build_in_maps(x, a_flat, b_flat):
    mts, wb = _host_factor(a_flat, b_flat)
    ident = np.eye(TILE, dtype=np.float32)
    return [
        {
            "x": np.ascontiguousarray(x[i * BC : (i + 1) * BC]),
            "mts": mts,
            "wb": wb,
            "ident": ident,
        }
        for i in range(NCORES)
    ]
